# revision 13
# baseline (speedup 1.0000x reference)
"""Contrastive-loss kernel for Trainium2 (8 NeuronCores, SPMD data-parallel).

Math (from the reference):
    diag_A_is = (A_is_t + A_is_t_14 + A_is_t_28)[i, i, :]        # [B, D]
    diag_A_em = (A_em_t + A_em_t_14 + A_em_t_28)[i, i, :]        # [B, D]
    loss = sum_b relu( sum_d (0.4*m + 0.6*tr_m) * (diag_A_is - diag_A_em) )

Only the diagonals A[i, i, :] of the six [B, B, D] tensors are touched
(1/256th of the data).  Sharding strategy: batch-dim data parallel across
the 8 cores — the host gathers the diagonal rows (pure data movement) and
ships each core its 32 rows of the eight [B, D] operands; all arithmetic
runs on-device.  Per-core partial losses are summed on the host (8
scalars).

Precision plan (tolerance 2e-2): m/tr_m ship as bf16 (rel err 2e-4); the
six A diagonals ship as fp8 e4m3 (rel err 4e-3, host-measured) and are
cast to bf16 in the SDMA datapath by gpsimd (SWDGE) casting DMAs — HBM
traffic is halved while the DVE still reads bf16 at full rate.  All
accumulation is fp32 (DVE accumulator / PSUM).

Device-side layout per core:
  wt [128, 288]  = m 0:256 | E 256:288      (sync HWDGE, bf16, 576 B rows)
  trt [128, 256] = tr_m                     (scalar HWDGE, bf16, 512 B rows)
  at [128, 1536] = is0|em0 | is1|em1 | is2|em2   (3 gpsimd casting DMAs,
                                                  fp8 -> bf16, FIFO)
  each [32, 1024] operand block is flattened row-major to [128, 256]
  (partition p = 4*row + quarter, 256 contiguous d's per partition).
  E[p, b] = 1.0 iff p // 4 == b — matmul rhs that folds the four
  per-partition quarter-row dots of each batch row (partition reduction).

Factoring: 0.4*m + 0.6*tr_m = 0.4 * (m + 1.5*tr_m) and
relu(0.4 x) = 0.4 relu(x), so the 0.4 is applied host-side to the scalar.
Linearity: sum_d w*(is-em) = sum_d [is|em] * [w|-w], so one
scalar_tensor_tensor per [is_i|em_i] chunk computes that chunk's dot
contribution in a single accumulator pass (rowq[:, i] = per-partition
quarter dots); the three chunks PSUM-accumulate through the E matmul.
Chunks 0/1 run on DVE; chunk 2 runs on GpSimd in parallel.

relu+sum runs on DVE (the Scalar engine's ACT path lazily loads a 1.3 us
function table on first use — measured, avoid).  The 4-byte result store
is issued from the sync ring.

Raw bass (no TileContext) on purpose: this walrus build enforces a tiny
per-instruction sync-wait limit and Tile's epilogue barrier costs several
microseconds.  Engines pipeline, so a same-engine consumer of an earlier
op's output still needs a semaphore edge (the race detector enforces it).
"""

import numpy as np
import ml_dtypes

import concourse.bass as bass
import concourse.mybir as mybir
from concourse.bass_utils import run_bass_kernel_spmd

B = 256
D = 1024
N_CORES = 8
ROWS_PER_CORE = B // N_CORES  # 32
BLK = 256  # free-dim width of one packed [32, 1024] operand block
E_COLS = ROWS_PER_CORE  # 32
FREE_W = BLK + E_COLS  # 288: m | E
FREE_A = 6 * BLK  # 1536: three [is|em] chunks
N_CHUNK = 3

_NC_CACHE = None


def build_nc() -> bass.Bass:
    f32 = mybir.dt.float32
    bf16 = mybir.dt.bfloat16
    f8 = mybir.dt.float8e4
    Alu = mybir.AluOpType

    nc = bass.Bass()
    xm = nc.dram_tensor("xm", [128 * FREE_W], bf16, kind="ExternalInput")
    xtr = nc.dram_tensor("xtr", [128 * BLK], bf16, kind="ExternalInput")
    xa = nc.dram_tensor("xa", [128 * FREE_A], f8, kind="ExternalInput")
    out_d = nc.dram_tensor("out", [1, 1], f32, kind="ExternalOutput")

    def xa_chunk(i):  # chunk-major flat layout: one contiguous DRAM range
        return xa[i * 128 * 2 * BLK : (i + 1) * 128 * 2 * BLK].rearrange(
            "(p f) -> p f", f=2 * BLK
        )

    with (
        nc.sbuf_tensor("wt", [128, FREE_W], bf16) as wt,
        nc.sbuf_tensor("trt", [128, BLK], bf16) as trt,
        nc.sbuf_tensor("at", [128, FREE_A], bf16) as at,
        nc.sbuf_tensor("w2", [128, 2 * BLK], bf16) as w2,
        nc.sbuf_tensor("prod", [128, FREE_A], bf16) as prod,
        nc.sbuf_tensor("rowq", [128, N_CHUNK], bf16) as rowq,
        nc.sbuf_tensor("srelu", [1, E_COLS], f32) as srelu,
        nc.sbuf_tensor("total", [1, 1], f32) as total,
        nc.psum_tensor("ps", [1, E_COLS], f32) as ps,
        nc.semaphore("sw") as sw,  # sync ring: m|E load (+16), out store (+16)
        nc.semaphore("st") as st,  # scalar ring: tr load
        nc.semaphore("s1") as s1,  # gpsimd ring: chunk 0
        nc.semaphore("s2") as s2,  # gpsimd ring: chunk 1
        nc.semaphore("s3") as s3,  # gpsimd ring: chunk 2
        nc.semaphore("vs") as vs,  # vector progress
        nc.semaphore("pe") as pe,  # tensor: partition fold done
        nc.Block() as block,
    ):
        m_ap = wt[:, 0:BLK]
        e_ap = wt[:, BLK:FREE_W]
        tr_ap = trt[:, :]
        chunk_sems = [s1, s2, s3]

        @block.sync
        def _(sync):
            sync.dma_start(
                out=wt[:, :], in_=xm[:].rearrange("(p f) -> p f", f=FREE_W)
            ).then_inc(sw, 16)
            sync.wait_ge(vs, 6)
            sync.dma_start(out=out_d[:], in_=total[:]).then_inc(sw, 16)
            sync.wait_ge(sw, 32)

        @block.scalar
        def _(scalar):
            scalar.dma_start(
                out=trt[:, :], in_=xtr[:].rearrange("(p f) -> p f", f=BLK)
            ).then_inc(st, 16)

        @block.gpsimd
        def _(gpsimd):
            # fp8 -> bf16 casting loads (SWDGE datapath cast), FIFO pipelined
            # (the Pool engine cannot run TensorScalarPtr — codegen rejects it,
            # so all three fused dots stay on DVE)
            for i in range(N_CHUNK):
                gpsimd.dma_start(
                    out=at[:, 2 * BLK * i : 2 * BLK * (i + 1)], in_=xa_chunk(i)
                ).then_inc(chunk_sems[i], 16)

        @block.vector
        def _(vector):
            # w = m + 1.5*tr_m  and  -w = (-1.5)*tr_m - m
            vector.wait_ge(sw, 16)
            vector.wait_ge(st, 16)
            nc.vector.scalar_tensor_tensor(
                out=w2[:, 0:BLK], in0=tr_ap, scalar=1.5, in1=m_ap,
                op0=Alu.mult, op1=Alu.add,
            ).then_inc(vs, 1)
            nc.vector.scalar_tensor_tensor(
                out=w2[:, BLK : 2 * BLK], in0=tr_ap, scalar=-1.5, in1=m_ap,
                op0=Alu.mult, op1=Alu.subtract,
            ).then_inc(vs, 1)
            vector.wait_ge(vs, 2)  # w2 committed (engines pipeline)
            # per chunk: fused dot prod = [is|em] * [w|-w];
            # rowq[:, i] = per-partition sum
            for i in range(N_CHUNK):
                vector.wait_ge(chunk_sems[i], 16)
                nc.vector.scalar_tensor_tensor(
                    out=prod[:, 2 * BLK * i : 2 * BLK * (i + 1)],
                    in0=at[:, 2 * BLK * i : 2 * BLK * (i + 1)],
                    scalar=1.0, in1=w2[:, :],
                    op0=Alu.mult, op1=Alu.mult,
                    accum_out=rowq[:, i : i + 1],
                ).then_inc(vs, 1)
            # relu the 32 per-row dots (in PSUM), accumulate to one scalar
            vector.wait_ge(pe, 1)
            nc.vector.tensor_scalar(
                out=srelu[:], in0=ps[:], scalar1=0.0, scalar2=None,
                op0=Alu.max, op1=Alu.add, accum_out=total[:],
            ).then_inc(vs, 1)

        @block.tensor
        def _(tensor):
            # ps[1, 32] += rowq[:, i]^T @ E — PSUM-accumulate the three
            # chunk dots while folding each row's 4 partition-quarters
            waits = [(vs, 3), (vs, 4), (vs, 5)]
            for i in range(N_CHUNK):
                tensor.wait_ge(*waits[i])
                mm = nc.tensor.matmul(
                    ps[:], rowq[:, i : i + 1], e_ap,
                    start=(i == 0), stop=(i == N_CHUNK - 1),
                )
                if i == N_CHUNK - 1:
                    mm.then_inc(pe, 1)

    return nc


def pack_inputs(A_is_t, A_is_t_14, A_is_t_28, A_em_t, A_em_t_14, A_em_t_28, m, tr_m):
    idx = np.arange(B)
    bf = ml_dtypes.bfloat16
    f8 = ml_dtypes.float8_e4m3fn

    def blk(a, dt):  # per-core [128, 256] flattening of a [B, D] operand
        return np.ascontiguousarray(a, dtype=np.float32).astype(dt).reshape(
            N_CORES, 128, BLK
        )

    def dblk(a):  # diagonal gather then per-core flatten, in fp8
        return blk(np.asarray(a)[idx, idx], f8)

    Xm = np.empty((N_CORES, 128, FREE_W), dtype=bf)
    Xm[:, :, 0:BLK] = blk(m, bf)
    Xm[:, :, BLK:FREE_W] = np.repeat(
        np.eye(E_COLS, dtype=np.float32), 4, axis=0
    ).astype(bf)
    Xtr = blk(tr_m, bf)

    Xa = np.empty((N_CORES, 128, FREE_A), dtype=f8)
    Xa[:, :, 0 * BLK : 1 * BLK] = dblk(A_is_t)
    Xa[:, :, 1 * BLK : 2 * BLK] = dblk(A_em_t)
    Xa[:, :, 2 * BLK : 3 * BLK] = dblk(A_is_t_14)
    Xa[:, :, 3 * BLK : 4 * BLK] = dblk(A_em_t_14)
    Xa[:, :, 4 * BLK : 5 * BLK] = dblk(A_is_t_28)
    Xa[:, :, 5 * BLK : 6 * BLK] = dblk(A_em_t_28)

    # xa is chunk-major: each 128x512 chunk is one contiguous DRAM range
    return [
        {
            "xm": Xm[c].ravel(),
            "xtr": Xtr[c].ravel(),
            "xa": np.concatenate(
                [Xa[c, :, 2 * BLK * i : 2 * BLK * (i + 1)].ravel()
                 for i in range(N_CHUNK)]
            ),
        }
        for c in range(N_CORES)
    ]


def run(in_maps, **kwargs):
    global _NC_CACHE
    if _NC_CACHE is None:
        _NC_CACHE = build_nc()
    return run_bass_kernel_spmd(
        _NC_CACHE, in_maps, core_ids=list(range(N_CORES)), **kwargs
    )


def kernel(**inputs) -> np.ndarray:
    res = run(pack_inputs(**inputs))
    total = 0.4 * sum(float(r["out"][0, 0]) for r in res.results)
    return np.array([total], dtype=np.float32)
